# revision 7
# baseline (speedup 1.0000x reference)
"""Trainium2 Bass kernel for the vq_codebook / HDC problem (v2).

Math (reference):
    hv      = sign(feat @ proj_w.T)                  [N=16384, D=10000], +-1 (0 -> +1)
    per_cls = segment_sum(hv, labels, K=3)           [3, D]
    updated = classify_weights + 0.5 * per_cls
    protos  = updated / max(||updated||_row, eps)
    logits  = hv @ protos.T                          [N, 3]

Strategy (8 NeuronCores, D-sharded, no collectives):
  * Each core owns DLOC=1250 hyper-dims (10 tiles of PT=125), all N rows.
    Host sorts rows by label so segment sums are contiguous-range sums.
  * mm1 in fp16 (sign flips from fp16 rounding contribute ~0.005 rel err,
    well under the 2e-2 budget), 1024-col moving chunks.
  * The PSUM->SBUF sign drain is the bottleneck: ACT (Sign, +-1 out) and
    DVE (is_ge, {0,1} out) each drain [125, 2048] psum tiles with fused
    per-segment accumulation.  Engine assignment rotates per superchunk
    between two tile-set patterns (6/4 and 5/5 split) so the faster ACT
    lane gets ~53% of the columns.
  * Finalize (per-tile class sums -> u2/2 -> fp8 hi/lo stationaries) runs
    on GPSIMD/ACT/DVE pipelined behind the drain; two stationary sets
    (one per drain pattern) with A-block (+-1 tiles) / B-block ({0,1})
    columns; host applies the 2x/colsum corrections per pattern.
  * mm2: fp8 DoubleRow matmuls, psum [12, MCH2] per chunk, drained by
    alternating ACT/DVE copies and DMAed out on alternating queues.
"""

import os
import sys

sys.path.insert(0, "/opt/trn_rl_repo")
os.environ.setdefault("MYCRO_LOCAL_CACHE", "1")

import numpy as np

import concourse.bass as bass
import concourse.tile as tile
from concourse import bacc
from concourse import mybir
from concourse.bass import MemorySpace
from concourse.bass_utils import run_bass_kernel_spmd

# ---------------------------------------------------------------- constants
N = 16384          # rows
C = 128            # feat dim (contraction)
D = 10000          # hyper dim
K = 3              # classes
NCORES = 8
DLOC = D // NCORES          # 1250 per core
PT = 125                    # partitions per d-tile
NT = DLOC // PT             # 10 d-tiles per core
NPAIR = NT // 2             # 5 fp8 DoubleRow tile pairs
SCH = 2048                  # mm1 superchunk columns (one psum tile, 4 banks)
NJ = N // SCH               # 8 superchunks
MCH = 512                   # mm1 matmul chunk columns (psum-bank output limit)
MCH2 = 512                  # mm2 output chunk columns (psum-bank output limit)
NC2 = N // MCH2             # 32 mm2 chunks
FP16 = mybir.dt.float16
FP8 = mybir.dt.float8e4

# Engine-assignment patterns: per superchunk j, tiles in ACT_SET drain on
# ACT (Sign, +-1); the rest on DVE (is_ge, {0,1}).  Two patterns balance
# the lanes (ACT is faster per column): S6 on 2 of 8 superchunks.
S6 = (0, 1, 2, 3, 4, 5)
S5 = (0, 1, 2, 3, 4)
SET_JS = {0: (0, 4)}                      # superchunks using pattern 0 (S6)
N_DUMMY = 36                # PE keep-warm matmuls over the finalize window

LAM = 0.5
EPS = 1e-12

LAST_RESULTS = None         # BassKernelResults of the most recent run (for test.py)


def make_plan(cuts):
    """Shared host/device plan derived from the sorted-label cuts.

    Returns (segs, act_set, set_id, nseg) where
      segs[j]   = [(s0, s1, cls), ...] segments of superchunk j
      act_set[j] = tuple of tiles drained on ACT for superchunk j
      set_id[j]  = stationary-set index (0 for S6 pattern, 1 for S5)
    Slot layout: spart column (t, slot) with slot enumerating (j, si).
    """
    segs = []
    for j in range(NJ):
        lo, hi = j * SCH, (j + 1) * SCH
        pts = [lo] + [b for b in cuts if lo < b < hi] + [hi]
        out = []
        for a, b in zip(pts[:-1], pts[1:]):
            cls = 0 if a < cuts[0] else (1 if a < cuts[1] else 2)
            out.append((a - lo, b - lo, cls))
        segs.append(out)
    act_set = [S6 if j in SET_JS[0] else S5 for j in range(NJ)]
    set_id = [0 if j in SET_JS[0] else 1 for j in range(NJ)]
    nseg = sum(len(s) for s in segs)
    return segs, act_set, set_id, nseg


def plan_slots(segs):
    """slot index for (j, si)."""
    slot = {}
    s = 0
    for j in range(NJ):
        for si in range(len(segs[j])):
            slot[(j, si)] = s
            s += 1
    return slot


def act_tiles_of_set(s):
    return S6 if s == 0 else S5


def hi_lo_slots(st_ap, t, in_a):
    """hi/lo AP slices in a stationary tile for tile t (A or B block)."""
    h = t % 2
    base = 0 if in_a else 2 * K
    hi = st_ap[:, h, base: base + K]
    lo = st_ap[:, h, base + K: base + 2 * K]
    return hi, lo


def build_nc(cuts):
    """Build the single-core Bass program (same for all cores; only DRAM
    inputs differ per core).  cuts = [c0, c0+c1] sorted-label boundaries."""
    segs, act_set, set_id, nseg = make_plan(cuts)
    slot = plan_slots(segs)

    nc = bacc.Bacc()
    featT = nc.dram_tensor("featT", [C, N], FP16, kind="ExternalInput")
    projwT = nc.dram_tensor("projwT", [C, DLOC], FP16, kind="ExternalInput")
    cwb = nc.dram_tensor("cwb", [PT, NT * K], mybir.dt.float32, kind="ExternalInput")
    p_out = nc.dram_tensor("p_out", [4 * K, N], mybir.dt.float32, kind="ExternalOutput")
    s_out = nc.dram_tensor("s_out", [PT, NT * 2 * K], mybir.dt.float32, kind="ExternalOutput")

    with tile.TileContext(nc) as tc:
        with (
            tc.tile_pool(name="singles", bufs=1) as singles,
            tc.tile_pool(name="feat", bufs=3) as featp,
            tc.tile_pool(name="u2f", bufs=2) as u2fp,
            tc.tile_pool(name="pstage", bufs=3) as pstp,
        ):
            # hv tiles first so their SBUF byte offsets stay 16B-aligned
            # (DoubleRow rhs requires 2B-aligned partition addresses)
            hv = [singles.tile([PT, 2, N], FP8, name=f"hv{p}") for p in range(NPAIR)]
            projw_sb = singles.tile([C, DLOC], FP16)
            # first feat quarter on the sync queue so mm1(0,0) starts early;
            # projw on the gpsimd queue in parallel
            fj0 = featp.tile([C, SCH], FP16, tag="fj")
            nc.sync.dma_start(out=fj0[:, :MCH], in_=featT[:, :MCH])
            nc.gpsimd.dma_start(out=projw_sb, in_=projwT[:, :])
            nc.sync.dma_start(out=fj0[:, MCH:], in_=featT[:, MCH:SCH])
            cwb_sb = singles.tile([PT, NT * K], mybir.dt.float32)
            spart = singles.tile([PT, NT * nseg], mybir.dt.float32)
            s2 = singles.tile([PT, NT * 2 * K], mybir.dt.float32)
            # DoubleRow lhsT outer free step must be 16B-aligned -> pad the
            # per-plane stationary stride to 16 columns
            stat = [
                [singles.tile([PT, 2, 16], FP8, name=f"st{s}_{p}") for p in range(NPAIR)]
                for s in range(2)
            ]
            dums = singles.tile([C, 256], mybir.dt.bfloat16)
            nc.gpsimd.dma_start(out=cwb_sb, in_=cwb[:, :])
            for s in range(2):
                for p in range(NPAIR):
                    nc.vector.memset(stat[s][p], 0.0)
            nc.vector.memset(dums, 0.0)

            # ---- produce: z psum chunks -> fp8 hv tiles + segment sums ----
            with tc.tile_pool(name="mm1ps", bufs=2, space=MemorySpace.PSUM) as mm1ps:
                for j in range(NJ):
                    if j == 0:
                        fj = fj0
                    else:
                        fj = featp.tile([C, SCH], FP16, tag="fj")
                        dma_eng = nc.gpsimd if j % 2 == 0 else nc.sync
                        dma_eng.dma_start(
                            out=fj, in_=featT[:, j * SCH:(j + 1) * SCH])
                    for t in range(NT):
                        ps = mm1ps.tile([PT, SCH], mybir.dt.float32, tag="mm1")
                        for h in range(SCH // MCH):
                            nc.tensor.matmul(
                                ps[:, h * MCH:(h + 1) * MCH],
                                projw_sb[:, t * PT:(t + 1) * PT],
                                fj[:, h * MCH:(h + 1) * MCH],
                                start=True, stop=True,
                            )
                        on_act = t in act_set[j]
                        for si, (s0, s1, _cls) in enumerate(segs[j]):
                            hv_sl = hv[t // 2][:, t % 2, j * SCH + s0: j * SCH + s1]
                            col = t * nseg + slot[(j, si)]
                            acc = spart[:, col: col + 1]
                            if on_act:
                                nc.scalar.activation(
                                    hv_sl, ps[:, s0:s1],
                                    mybir.ActivationFunctionType.Sign,
                                    accum_out=acc,
                                )
                            else:
                                # {0,1} in one op; accum = count of positives
                                # (op1 is the accum reduce op, not elementwise)
                                nc.vector.tensor_scalar(
                                    hv_sl, ps[:, s0:s1], 0.0, None,
                                    mybir.AluOpType.is_ge, mybir.AluOpType.add,
                                    accum_out=acc,
                                )

            # ---- per-tile finalize: sA/sB class sums, u2/2, fp8 hi/lo ----
            # GPSIMD chains the spart partial sums (it is otherwise idle);
            # ACT casts hi to fp8, DVE computes lo; GPSIMD replicates both
            # into the second stationary set.
            for t in range(NT):
                for k in range(K):
                    acols = [t * nseg + slot[(j, si)]
                             for j in range(NJ)
                             for si, (_a, _b, cls) in enumerate(segs[j])
                             if cls == k and t in act_set[j]]
                    dcols = [t * nseg + slot[(j, si)]
                             for j in range(NJ)
                             for si, (_a, _b, cls) in enumerate(segs[j])
                             if cls == k and t not in act_set[j]]
                    for grp, cols in ((0, acols), (1, dcols)):
                        dst = s2[:, t * 2 * K + grp * K + k: t * 2 * K + grp * K + k + 1]
                        if not cols:
                            nc.gpsimd.memset(dst, 0.0)
                        elif len(cols) == 1:
                            nc.gpsimd.tensor_copy(dst, spart[:, cols[0]:cols[0] + 1])
                        else:
                            nc.gpsimd.tensor_tensor(
                                dst, spart[:, cols[0]:cols[0] + 1],
                                spart[:, cols[1]:cols[1] + 1], mybir.AluOpType.add)
                            for cidx in cols[2:]:
                                nc.gpsimd.tensor_tensor(
                                    dst, dst, spart[:, cidx:cidx + 1],
                                    mybir.AluOpType.add)
                # u2 = cwb2 + sA + 2*sB  (cwb2 = 2cw - len_dve; Pool has no
                # scalar ops, so build the full u2 with adds and fold the
                # /2 into the ACT fp8 cast's scale)
                u2f = u2fp.tile([PT, K], mybir.dt.float32, tag="u2f")
                sA = s2[:, t * 2 * K: t * 2 * K + K]
                sB = s2[:, t * 2 * K + K: t * 2 * K + 2 * K]
                nc.gpsimd.tensor_tensor(u2f, sB, sB, mybir.AluOpType.add)
                nc.gpsimd.tensor_tensor(u2f, u2f, sA, mybir.AluOpType.add)
                nc.gpsimd.tensor_tensor(
                    u2f, u2f, cwb_sb[:, t * K:(t + 1) * K], mybir.AluOpType.add)
                in_a0 = t in act_tiles_of_set(0)
                in_a1 = t in act_tiles_of_set(1)
                hi0, lo0 = hi_lo_slots(stat[0][t // 2], t, in_a0)
                hi1, lo1 = hi_lo_slots(stat[1][t // 2], t, in_a1)
                nc.scalar.activation(
                    hi0, u2f, mybir.ActivationFunctionType.Copy, scale=0.5)
                nc.vector.scalar_tensor_tensor(
                    lo0, u2f, 0.5, hi0,
                    mybir.AluOpType.mult, mybir.AluOpType.subtract,
                )
                nc.gpsimd.tensor_copy(hi1, hi0)
                nc.gpsimd.tensor_copy(lo1, lo0)

            nc.gpsimd.dma_start(out=s_out[:, :], in_=s2)

            with tc.tile_pool(name="pps", bufs=3, space=MemorySpace.PSUM) as pps:
                # ---- PE keep-warm bridge over the drain/finalize tail ------
                for i in range(N_DUMMY):
                    dpp = pps.tile([128, MCH2], mybir.dt.float32, tag="pp")
                    nc.tensor.matmul(
                        dpp[:PT, :256], dums[:, 0:PT], dums,
                        start=True, stop=True,
                    )

                # ---- mm2: P2 partials via fp8 DoubleRow -------------------
                pst = None
                for c in range(NC2):
                    sid = set_id[(c * MCH2) // SCH]
                    pp = pps.tile([128, MCH2], mybir.dt.float32, tag="pp")
                    for p in range(NPAIR):
                        nc.tensor.matmul(
                            pp[:4 * K, :], stat[sid][p][:, :, 0:4 * K],
                            hv[p][:, :, c * MCH2:(c + 1) * MCH2],
                            start=(p == 0), stop=(p == NPAIR - 1),
                            perf_mode=mybir.MatmulPerfMode.DoubleRow,
                        )
                    if c % 2 == 0:
                        pst = pstp.tile([4 * K, 2 * MCH2], mybir.dt.float32, tag="pst")
                    half = pst[:, (c % 2) * MCH2:(c % 2 + 1) * MCH2]
                    # only ACT/DVE can read PSUM
                    if c % 2 == 0:
                        nc.vector.tensor_copy(half, pp[:4 * K, :])
                    else:
                        nc.scalar.activation(
                            half, pp[:4 * K, :], mybir.ActivationFunctionType.Copy
                        )
                    if c % 2 == 1:
                        dma_eng = nc.gpsimd if (c // 2) % 2 == 0 else nc.sync
                        dma_eng.dma_start(
                            out=p_out[:, (c - 1) * MCH2:(c + 1) * MCH2],
                            in_=pst,
                        )
    nc.compile()
    return nc


def _prep_inputs(feat_s, proj_w, classify_weights, cuts):
    segs, act_set, _set_id, _nseg = make_plan(cuts)
    # len_dve(t, k): total {0,1}-segment length of class k for tile t
    len_dve = np.zeros((NT, K), np.float32)
    for j in range(NJ):
        for (a, b, cls) in segs[j]:
            for t in range(NT):
                if t not in act_set[j]:
                    len_dve[t, cls] += b - a

    featT = np.ascontiguousarray(feat_s.T).astype(np.float16)  # [128, N]
    in_maps = []
    for core in range(NCORES):
        sl = slice(core * DLOC, (core + 1) * DLOC)
        projwT = np.ascontiguousarray(proj_w[sl].T).astype(np.float16)  # [128, DLOC]
        # device builds u2 = cwb2 + sA + 2*sB with cwb2 = 2cw - len_dve
        cw2 = classify_weights[:, sl].astype(np.float32).T              # [DLOC, 3]
        cw2 = 2.0 * cw2.reshape(NT, PT, K) - len_dve[:, None, :]
        cwbv = np.ascontiguousarray(
            cw2.transpose(1, 0, 2).reshape(PT, NT * K)
        )
        in_maps.append({"featT": featT, "projwT": projwT, "cwb": cwbv})
    return in_maps


def _fp8_round(x):
    import ml_dtypes
    return x.astype(ml_dtypes.float8_e4m3fn).astype(np.float64)


def kernel(feat, proj_w, classify_weights, labels, _trace=False):
    global LAST_RESULTS
    feat = np.asarray(feat, dtype=np.float32)
    proj_w = np.asarray(proj_w, dtype=np.float32)
    classify_weights = np.asarray(classify_weights, dtype=np.float32)
    labels = np.asarray(labels).astype(np.int64)

    perm = np.argsort(labels, kind="stable")
    feat_s = feat[perm]
    counts = np.bincount(labels, minlength=K)
    cuts = [int(counts[0]), int(counts[0] + counts[1])]

    nc = build_nc(cuts)
    in_maps = _prep_inputs(feat_s, proj_w, classify_weights, cuts)
    res = run_bass_kernel_spmd(nc, in_maps, list(range(NCORES)), trace=_trace)
    LAST_RESULTS = res

    segs, act_set, set_id, _nseg = make_plan(cuts)
    len_dve = np.zeros((NT, K), np.float64)
    for j in range(NJ):
        for (a, b, cls) in segs[j]:
            for t in range(NT):
                if t not in act_set[j]:
                    len_dve[t, cls] += b - a

    S = np.zeros((K, D), np.float64)
    hvu2 = np.zeros((K, N), np.float64)
    # csB corrections use the quantized u2/2 (matches device stationaries)
    for core in range(NCORES):
        s_raw = np.asarray(res.results[core]["s_out"]).astype(np.float64)  # [PT, NT*2K]
        s_raw = s_raw.reshape(PT, NT, 2, K)
        sA = s_raw[:, :, 0, :]                       # [PT, NT, K]
        sB = s_raw[:, :, 1, :]
        S_core = sA + 2.0 * sB - len_dve[None, :, :]  # [PT, NT, K]
        for t in range(NT):
            S[:, core * DLOC + t * PT: core * DLOC + (t + 1) * PT] = S_core[:, t, :].T

    updated = classify_weights.astype(np.float64) + LAM * S   # [K, D]
    norms = np.linalg.norm(updated, axis=1)
    u2 = 2.0 * classify_weights.astype(np.float64) + S        # [K, D]
    # quantized u2/2 as the device stationaries hold it
    u2h_hi = _fp8_round(u2 / 2.0)
    u2h = u2h_hi + _fp8_round(u2 / 2.0 - u2h_hi)

    # per-core, per-tile colsums of quantized u2 for the B-block fixup
    csB = np.zeros((2, K), np.float64)
    for s in range(2):
        a_tiles = act_tiles_of_set(s)
        for core in range(NCORES):
            for t in range(NT):
                if t not in a_tiles:
                    dsl = slice(core * DLOC + t * PT, core * DLOC + (t + 1) * PT)
                    csB[s] += 2.0 * u2h[:, dsl].sum(axis=1)

    for core in range(NCORES):
        p = np.asarray(res.results[core]["p_out"]).astype(np.float64)   # [12, N]
        hvu2 += 2.0 * (p[0:3] + p[3:6]) + 4.0 * (p[6:9] + p[9:12])
    for c in range(NC2):
        sid = set_id[(c * MCH2) // SCH]
        hvu2[:, c * MCH2:(c + 1) * MCH2] -= csB[sid][:, None]

    logits_sorted = (hvu2 / (2.0 * np.maximum(norms, EPS))[:, None]).T.astype(np.float32)
    out = np.empty((N, K), np.float32)
    out[perm] = logits_sorted
    return out


# revision 10
# speedup vs baseline: 1.1642x; 1.1642x over previous
"""Trainium2 Bass kernel for the vq_codebook / HDC problem (v3).

Math (reference):
    hv      = sign(feat @ proj_w.T)                  [N=16384, D=10000], +-1 (0 -> +1)
    per_cls = segment_sum(hv, labels, K=3)           [3, D]
    updated = classify_weights + 0.5 * per_cls
    protos  = updated / max(||updated||_row, eps)
    logits  = hv @ protos.T                          [N, 3]

Strategy (8 NeuronCores, D-sharded, no collectives):
  * Each core owns DLOC=1250 hyper-dims (10 tiles of PT=125), all N rows.
    Host sorts rows by label so segment sums are contiguous-range sums.
  * mm1 in fp16 (sign flips from fp16 rounding contribute ~0.005 rel err,
    well under the 2e-2 budget); psum tiles are [125, 2048] (4 banks x2).
  * The PSUM->SBUF sign drain is the bottleneck: ACT (Sign, +-1 out) and
    DVE (is_ge, {0,1} out) each drain [125, 2048] ops with fused
    per-segment accumulation.  Tile->engine assignment is INTERLEAVED
    (even tiles ACT, odd DVE) so consecutive drain rounds alternate
    engines and the PE never idles past the HAM window; two patterns
    (S6 adds tile 9 to ACT on 2 of 8 superchunks) balance the lanes.
  * Finalize is a handful of WIDE strided ops: Pool accumulates the
    per-segment accums into cwb (ACT groups) / u2d ({0,1} groups, later
    doubled), then u2 = cwb + 2*u2d; ACT/DVE cast fp8 hi/lo directly
    into both pattern-stationary layouts.  Group adds are emitted two
    superchunks late so the Pool FIFO never head-of-line blocks.
  * mm2: fp8 DoubleRow matmuls, [12, 512] psum chunks, drained by
    alternating ACT/DVE copies and DMAed out on alternating queues.
    Host combines with per-pattern {0,1}-block corrections.
"""

import os
import sys

sys.path.insert(0, "/opt/trn_rl_repo")
os.environ.setdefault("MYCRO_LOCAL_CACHE", "1")

import numpy as np

import concourse.bass as bass
import concourse.tile as tile
from concourse import bacc
from concourse import mybir
from concourse.bass import MemorySpace
from concourse.bass_utils import run_bass_kernel_spmd

# ---------------------------------------------------------------- constants
N = 16384          # rows
C = 128            # feat dim (contraction)
D = 10000          # hyper dim
K = 3              # classes
NCORES = 8
DLOC = D // NCORES          # 1250 per core
PT = 125                    # partitions per d-tile
NT = DLOC // PT             # 10 d-tiles per core
NPAIR = NT // 2             # 5 fp8 DoubleRow tile pairs
SCH = 2048                  # mm1 superchunk columns (one psum tile, 4 banks)
NJ = N // SCH               # 8 superchunks
MCH = 512                   # mm1 matmul chunk columns (psum-bank output limit)
MCH2 = 512                  # mm2 output chunk columns (psum-bank output limit)
NC2 = N // MCH2             # 32 mm2 chunks
FP16 = mybir.dt.float16
FP8 = mybir.dt.float8e4

# Engine-assignment patterns: per superchunk j, tiles in the ACT set drain
# on ACT (Sign, +-1); the rest on DVE (is_ge, {0,1}).  Interleaved so
# consecutive (j, t) rounds alternate engines; S6 gives ACT an extra tile
# on 2 of 8 superchunks to balance the lanes (ACT is faster per column).
S6 = (0, 2, 4, 6, 8, 9)
S5 = (0, 2, 4, 6, 8)
SET0_JS = (0, 4)            # superchunks using pattern 0 (S6)
N_DUMMY = 30                # PE keep-warm matmuls over the drain tail

LAM = 0.5
EPS = 1e-12

LAST_RESULTS = None         # BassKernelResults of the most recent run (for test.py)


def make_plan(cuts):
    """segs[j] = [(s0, s1, cls), ...]; act_set[j]; set_id[j]; nseg."""
    segs = []
    for j in range(NJ):
        lo, hi = j * SCH, (j + 1) * SCH
        pts = [lo] + [b for b in cuts if lo < b < hi] + [hi]
        out = []
        for a, b in zip(pts[:-1], pts[1:]):
            cls = 0 if a < cuts[0] else (1 if a < cuts[1] else 2)
            out.append((a - lo, b - lo, cls))
        segs.append(out)
    act_set = [S6 if j in SET0_JS else S5 for j in range(NJ)]
    set_id = [0 if j in SET0_JS else 1 for j in range(NJ)]
    nseg = sum(len(s) for s in segs)
    return segs, act_set, set_id, nseg


def plan_slots(segs):
    slot = {}
    s = 0
    for j in range(NJ):
        for si in range(len(segs[j])):
            slot[(j, si)] = s
            s += 1
    return slot


def len_dve_table(segs, act_set):
    """len_dve[t, k]: total {0,1}-segment length of class k for tile t."""
    ld = np.zeros((NT, K), np.float64)
    for j in range(NJ):
        for (a, b, cls) in segs[j]:
            for t in range(NT):
                if t not in act_set[j]:
                    ld[t, cls] += b - a
    return ld


def build_nc(cuts):
    segs, act_set, set_id, nseg = make_plan(cuts)
    slot = plan_slots(segs)

    nc = bacc.Bacc()
    featT = nc.dram_tensor("featT", [C, N], FP16, kind="ExternalInput")
    projwT = nc.dram_tensor("projwT", [C, DLOC], FP16, kind="ExternalInput")
    # cwb = 2*cw - len_dve, laid out [PT, NT, K]
    cwb = nc.dram_tensor("cwb", [PT, NT * K], mybir.dt.float32, kind="ExternalInput")
    p_out = nc.dram_tensor("p_out", [4 * K, N], mybir.dt.float32, kind="ExternalOutput")
    # u2 = cwb + sA + 2*sB, fp32, [PT, NT, K] -> host recovers S = u2 - 2cw
    u2_out = nc.dram_tensor("u2_out", [PT, NT * K], mybir.dt.float32, kind="ExternalOutput")

    def emit_groups(j):
        """Pool group-adds for superchunk j's accum slots (ACT->cwb3,
        {0,1}->u2d3).  All strided-wide ops."""
        for si in range(len(segs[j])):
            s = slot[(j, si)]
            cls = segs[j][si][2]
            base = s * NT
            # ACT tiles: evens (both patterns)
            nc.gpsimd.tensor_tensor(
                cwb3[:, 0:NT:2, cls], cwb3[:, 0:NT:2, cls],
                spart[:, base + 0: base + NT: 2], mybir.AluOpType.add)
            if act_set[j] is S6:
                nc.gpsimd.tensor_tensor(
                    cwb3[:, NT - 1, cls: cls + 1], cwb3[:, NT - 1, cls: cls + 1],
                    spart[:, base + NT - 1: base + NT], mybir.AluOpType.add)
                nc.gpsimd.tensor_tensor(
                    u2d3[:, 1:NT - 1:2, cls], u2d3[:, 1:NT - 1:2, cls],
                    spart[:, base + 1: base + NT - 1: 2], mybir.AluOpType.add)
            else:
                nc.gpsimd.tensor_tensor(
                    u2d3[:, 1:NT:2, cls], u2d3[:, 1:NT:2, cls],
                    spart[:, base + 1: base + NT: 2], mybir.AluOpType.add)

    with tile.TileContext(nc) as tc:
        with (
            tc.tile_pool(name="singles", bufs=1) as singles,
            tc.tile_pool(name="feat", bufs=3) as featp,
            tc.tile_pool(name="pstage", bufs=3) as pstp,
        ):
            # hv tiles first so their SBUF byte offsets stay 16B-aligned
            # (DoubleRow rhs requires 2B-aligned partition addresses)
            hv = [singles.tile([PT, 2, N], FP8, name=f"hv{p}") for p in range(NPAIR)]
            projw_sb = singles.tile([C, DLOC], FP16)
            # first projw tile + feat quarter first so mm1(0,0) starts early
            fj0 = featp.tile([C, SCH], FP16, tag="fj")
            nc.gpsimd.dma_start(out=projw_sb[:, :PT], in_=projwT[:, :PT])
            nc.sync.dma_start(out=fj0[:, :MCH], in_=featT[:, :MCH])
            nc.gpsimd.dma_start(out=projw_sb[:, PT:], in_=projwT[:, PT:])
            nc.sync.dma_start(out=fj0[:, MCH:], in_=featT[:, MCH:SCH])
            # 2D tiles for DMA; 3D/4D rearranged views for strided compute
            # (flatten()/multi-dim DMA APs fail NEFF load)
            cwb2 = singles.tile([PT, NT * K], mybir.dt.float32)
            u2d2 = singles.tile([PT, NT * K], mybir.dt.float32)
            u2f2 = singles.tile([PT, NT * K], mybir.dt.float32)
            cwb3 = cwb2.rearrange("p (t k) -> p t k", t=NT)
            u2d3 = u2d2.rearrange("p (t k) -> p t k", t=NT)
            u2f3 = u2f2.rearrange("p (t k) -> p t k", t=NT)
            spart = singles.tile([PT, nseg * NT], mybir.dt.float32)
            # stationaries: [PT, half, pair, 16] fp8 view; cols 0:3 A-hi,
            # 3:6 A-lo, 6:9 B-hi, 9:12 B-lo, 12:16 zero pad.  mm2 lhsT slice
            # stat4[:, :, p, 0:12] has outer free step 80B (16B-aligned).
            stat2 = [singles.tile([PT, 2 * NPAIR * 16], FP8, name=f"st{s}")
                     for s in range(2)]
            stat = [s2.rearrange("p (h q c) -> p h q c", h=2, q=NPAIR)
                    for s2 in stat2]
            dums = singles.tile([C, 256], mybir.dt.bfloat16)
            nc.gpsimd.dma_start(out=cwb2, in_=cwb[:, :])
            nc.vector.memset(u2d2, 0.0)
            for s in range(2):
                nc.vector.memset(stat2[s], 0.0)
            nc.vector.memset(dums, 0.0)

            # ---- produce: z psum chunks -> fp8 hv tiles + segment sums ----
            with tc.tile_pool(name="mm1ps", bufs=2, space=MemorySpace.PSUM) as mm1ps:
                for j in range(NJ):
                    if j == 0:
                        fj = fj0
                    else:
                        fj = featp.tile([C, SCH], FP16, tag="fj")
                        dma_eng = nc.gpsimd if j % 2 == 0 else nc.sync
                        dma_eng.dma_start(
                            out=fj, in_=featT[:, j * SCH:(j + 1) * SCH])
                    if j >= 2:
                        emit_groups(j - 2)
                    for t in range(NT):
                        ps = mm1ps.tile([PT, SCH], mybir.dt.float32, tag="mm1")
                        for h in range(SCH // MCH):
                            nc.tensor.matmul(
                                ps[:, h * MCH:(h + 1) * MCH],
                                projw_sb[:, t * PT:(t + 1) * PT],
                                fj[:, h * MCH:(h + 1) * MCH],
                                start=True, stop=True,
                            )
                        on_act = t in act_set[j]
                        for si, (s0, s1, _cls) in enumerate(segs[j]):
                            hv_sl = hv[t // 2][:, t % 2, j * SCH + s0: j * SCH + s1]
                            col = slot[(j, si)] * NT + t
                            acc = spart[:, col: col + 1]
                            if on_act:
                                nc.scalar.activation(
                                    hv_sl, ps[:, s0:s1],
                                    mybir.ActivationFunctionType.Sign,
                                    accum_out=acc,
                                )
                            else:
                                # {0,1} in one op; accum = count of positives
                                # (op1 is the accum reduce op, not elementwise)
                                nc.vector.tensor_scalar(
                                    hv_sl, ps[:, s0:s1], 0.0, None,
                                    mybir.AluOpType.is_ge, mybir.AluOpType.add,
                                    accum_out=acc,
                                )

            # ---- finalize: u2 = cwb + sA + 2*sB; fp8 hi/lo into both sets --
            emit_groups(NJ - 2)
            emit_groups(NJ - 1)
            nc.gpsimd.tensor_tensor(u2f2, cwb2, u2d2, mybir.AluOpType.add)
            nc.gpsimd.tensor_tensor(u2f2, u2f2, u2d2, mybir.AluOpType.add)
            nc.gpsimd.dma_start(out=u2_out[:, :], in_=u2f2)

            # wide fp8 casts: hi = fp8(u2/2) on ACT, lo = fp8(u2/2 - hi) on DVE
            # (tile subset, half, pair-slice, in A block?) per stationary set
            Copy = mybir.ActivationFunctionType.Copy

            def cast_hi_lo(st, h, psl, u2sl, in_a):
                b = 0 if in_a else 2 * K
                hi = st[:, h, psl, b: b + K]
                lo = st[:, h, psl, b + K: b + 2 * K]
                nc.scalar.activation(hi, u2sl, Copy, scale=0.5)
                nc.vector.scalar_tensor_tensor(
                    lo, u2sl, 0.5, hi,
                    mybir.AluOpType.mult, mybir.AluOpType.subtract,
                )

            ev = slice(0, NT, 2)
            # pattern 0 (S6): A = evens + t9; B = {1,3,5,7}
            cast_hi_lo(stat[0], 0, slice(0, NPAIR), u2f3[:, ev, :], True)
            cast_hi_lo(stat[0], 1, NPAIR - 1, u2f3[:, NT - 1, :], True)
            cast_hi_lo(stat[0], 1, slice(0, NPAIR - 1), u2f3[:, 1:NT - 1:2, :], False)
            # pattern 1 (S5): A = evens; B = odds
            cast_hi_lo(stat[1], 0, slice(0, NPAIR), u2f3[:, ev, :], True)
            cast_hi_lo(stat[1], 1, slice(0, NPAIR), u2f3[:, 1:NT:2, :], False)

            with tc.tile_pool(name="pps", bufs=4, space=MemorySpace.PSUM) as pps:
                # ---- PE keep-warm bridge over the drain/finalize tail ------
                for i in range(N_DUMMY):
                    dpp = pps.tile([128, MCH2], mybir.dt.float32, tag="pp")
                    nc.tensor.matmul(
                        dpp[:PT, :256], dums[:, 0:PT], dums,
                        start=True, stop=True,
                    )

                # ---- mm2: P2 partials via fp8 DoubleRow -------------------
                pst = None
                for c in range(NC2):
                    sid = set_id[(c * MCH2) // SCH]
                    pp = pps.tile([128, MCH2], mybir.dt.float32, tag="pp")
                    for p in range(NPAIR):
                        nc.tensor.matmul(
                            pp[:4 * K, :], stat[sid][:, :, p, 0:4 * K],
                            hv[p][:, :, c * MCH2:(c + 1) * MCH2],
                            start=(p == 0), stop=(p == NPAIR - 1),
                            perf_mode=mybir.MatmulPerfMode.DoubleRow,
                        )
                    if c % 2 == 0:
                        pst = pstp.tile([4 * K, 2 * MCH2], mybir.dt.float32, tag="pst")
                    half = pst[:, (c % 2) * MCH2:(c % 2 + 1) * MCH2]
                    # only ACT/DVE can read PSUM
                    if c % 2 == 0:
                        nc.vector.tensor_copy(half, pp[:4 * K, :])
                    else:
                        nc.scalar.activation(half, pp[:4 * K, :], Copy)
                    if c % 2 == 1:
                        dma_eng = nc.gpsimd if (c // 2) % 2 == 0 else nc.sync
                        dma_eng.dma_start(
                            out=p_out[:, (c - 1) * MCH2:(c + 1) * MCH2],
                            in_=pst,
                        )
    nc.compile()
    return nc


def _prep_inputs(feat_s, proj_w, classify_weights, cuts):
    segs, act_set, _set_id, _nseg = make_plan(cuts)
    ld = len_dve_table(segs, act_set)

    featT = np.ascontiguousarray(feat_s.T).astype(np.float16)  # [128, N]
    in_maps = []
    for core in range(NCORES):
        sl = slice(core * DLOC, (core + 1) * DLOC)
        projwT = np.ascontiguousarray(proj_w[sl].T).astype(np.float16)  # [128, DLOC]
        cw2 = classify_weights[:, sl].astype(np.float64).T              # [DLOC, 3]
        cw2 = 2.0 * cw2.reshape(NT, PT, K) - ld[:, None, :]
        cwbv = np.ascontiguousarray(
            cw2.transpose(1, 0, 2).reshape(PT, NT * K)
        ).astype(np.float32)
        in_maps.append({"featT": featT, "projwT": projwT, "cwb": cwbv})
    return in_maps


def _fp8_round(x):
    import ml_dtypes
    return x.astype(ml_dtypes.float8_e4m3fn).astype(np.float64)


def kernel(feat, proj_w, classify_weights, labels, _trace=False):
    global LAST_RESULTS
    feat = np.asarray(feat, dtype=np.float32)
    proj_w = np.asarray(proj_w, dtype=np.float32)
    classify_weights = np.asarray(classify_weights, dtype=np.float32)
    labels = np.asarray(labels).astype(np.int64)

    perm = np.argsort(labels, kind="stable")
    feat_s = feat[perm]
    counts = np.bincount(labels, minlength=K)
    cuts = [int(counts[0]), int(counts[0] + counts[1])]

    nc = build_nc(cuts)
    in_maps = _prep_inputs(feat_s, proj_w, classify_weights, cuts)
    res = run_bass_kernel_spmd(nc, in_maps, list(range(NCORES)), trace=_trace)
    LAST_RESULTS = res

    segs, act_set, set_id, _nseg = make_plan(cuts)

    # device u2 (fp32, exactly what the stationaries were quantized from)
    u2dev = np.zeros((K, D), np.float64)
    for core in range(NCORES):
        u2raw = np.asarray(res.results[core]["u2_out"]).astype(np.float64)
        u2raw = u2raw.reshape(PT, NT, K)
        for t in range(NT):
            dsl = slice(core * DLOC + t * PT, core * DLOC + (t + 1) * PT)
            u2dev[:, dsl] = u2raw[:, t, :].T

    S = u2dev - 2.0 * classify_weights.astype(np.float64)
    updated = classify_weights.astype(np.float64) + LAM * S
    norms = np.linalg.norm(updated, axis=1)

    # quantized u2/2 exactly as the device stationaries hold it
    hi = _fp8_round(u2dev / 2.0)
    q = hi + _fp8_round(u2dev / 2.0 - hi)

    # per-pattern {0,1}-block colsum corrections
    b_tiles = [tuple(t for t in range(NT) if t not in S6),
               tuple(t for t in range(NT) if t not in S5)]
    csB = np.zeros((2, K), np.float64)
    for s in range(2):
        for core in range(NCORES):
            for t in b_tiles[s]:
                dsl = slice(core * DLOC + t * PT, core * DLOC + (t + 1) * PT)
                csB[s] += 2.0 * q[:, dsl].sum(axis=1)

    hvu2 = np.zeros((K, N), np.float64)
    for core in range(NCORES):
        p = np.asarray(res.results[core]["p_out"]).astype(np.float64)   # [12, N]
        hvu2 += 2.0 * (p[0:3] + p[3:6]) + 4.0 * (p[6:9] + p[9:12])
    for c in range(NC2):
        sid = set_id[(c * MCH2) // SCH]
        hvu2[:, c * MCH2:(c + 1) * MCH2] -= csB[sid][:, None]

    logits_sorted = (hvu2 / (2.0 * np.maximum(norms, EPS))[:, None]).T.astype(np.float32)
    out = np.empty((N, K), np.float32)
    out[perm] = logits_sorted
    return out


# revision 12
# speedup vs baseline: 1.4934x; 1.2828x over previous
"""Trainium2 Bass kernel for the vq_codebook / HDC problem (v4).

Math (reference):
    hv      = sign(feat @ proj_w.T)                  [N=16384, D=10000], +-1 (0 -> +1)
    per_cls = segment_sum(hv, labels, K=3)           [3, D]
    updated = classify_weights + 0.5 * per_cls
    protos  = updated / max(||updated||_row, eps)
    logits  = hv @ protos.T                          [N, 3]

Strategy (8 NeuronCores, D-sharded, no collectives):
  * Each core owns DLOC=1250 hyper-dims (10 tiles of PT=125), all N rows.
    Host sorts rows by label so segment sums are contiguous-range sums.
  * mm1 in fp16 (sign flips from fp16 rounding cost ~0.005 rel err, well
    under the 2e-2 budget); psum [125, 1024] x 4 bufs -- enough pipeline
    slack that neither the PE pstate nor the drain->fill latency loop
    paces the phase (2048-col drains are cheaper per column but 2 psum
    bufs serialize drain and refill, measured 1.79us/round).
  * The PSUM->SBUF sign drain is the bottleneck: even tiles drain on ACT
    (Sign, +-1), odd on DVE (is_ge, {0,1}), with fused per-segment
    accumulation; tile 8 moves to DVE on 2 of 16 superchunks to balance
    the lanes.  Engines therefore alternate round by round.
  * Finalize is a handful of wide strided ops: Pool accumulates segment
    accums into cwb (ACT groups) / u2d ({0,1} groups), u2 = cwb + 2*u2d,
    then ACT/DVE cast fp8 hi/lo directly into both pattern-stationary
    sets.  Group adds are emitted two superchunks late so the Pool FIFO
    never head-of-line blocks.
  * mm2: fp8 DoubleRow matmuls, pair-outer over groups of 8 live
    [12, 512] psum chunks so LDWEIGHTS is hoisted (5 loads per group
    instead of 40; LDW costs ~110ns of PE issue otherwise).  Drained by
    alternating ACT/DVE copies, DMAed on alternating queues; host
    applies per-pattern {0,1}-block corrections.
"""

import os
import sys

sys.path.insert(0, "/opt/trn_rl_repo")
os.environ.setdefault("MYCRO_LOCAL_CACHE", "1")

import numpy as np

import concourse.bass as bass
import concourse.tile as tile
from concourse import bacc
from concourse import mybir
from concourse.bass import MemorySpace
from concourse.bass_utils import run_bass_kernel_spmd

# ---------------------------------------------------------------- constants
N = 16384          # rows
C = 128            # feat dim (contraction)
D = 10000          # hyper dim
K = 3              # classes
NCORES = 8
DLOC = D // NCORES          # 1250 per core
PT = 125                    # partitions per d-tile
NT = DLOC // PT             # 10 d-tiles per core
NPAIR = NT // 2             # 5 fp8 DoubleRow tile pairs
SCH = 1024                  # mm1 superchunk columns (one psum tile, 2 banks)
NJ = N // SCH               # 16 superchunks
MCH = 512                   # mm1 matmul chunk columns (psum-bank output limit)
MCH2 = 512                  # mm2 output chunk columns (psum-bank output limit)
NC2 = N // MCH2             # 32 mm2 chunks
GRP2 = 8                    # mm2 chunks per pair-outer group (8 psum banks)
FP16 = mybir.dt.float16
FP8 = mybir.dt.float8e4

# Engine-assignment patterns per superchunk: tiles in the ACT set drain on
# ACT (Sign, +-1); the rest on DVE (is_ge, {0,1}).  Interleaved so rounds
# alternate engines; pattern 0 (2 of 16 superchunks) moves tile 8 to DVE
# to shave the ACT lane (ACT is faster per column, slower per op).
S_EVEN = (0, 2, 4, 6, 8)
S4 = (0, 2, 4, 6)
SET0_JS = (0, 1)            # superchunks using pattern 0 (S4); also first
                            # mm2 group -> fewer LDW swaps
N_DUMMY = 18                # PE keep-warm matmuls over the drain tail

LAM = 0.5
EPS = 1e-12

LAST_RESULTS = None         # BassKernelResults of the most recent run (for test.py)


def act_tiles_of_set(s):
    return S4 if s == 0 else S_EVEN


def make_plan(cuts):
    """segs[j] = [(s0, s1, cls), ...]; act_set[j]; set_id[j]; nseg."""
    segs = []
    for j in range(NJ):
        lo, hi = j * SCH, (j + 1) * SCH
        pts = [lo] + [b for b in cuts if lo < b < hi] + [hi]
        out = []
        for a, b in zip(pts[:-1], pts[1:]):
            cls = 0 if a < cuts[0] else (1 if a < cuts[1] else 2)
            out.append((a - lo, b - lo, cls))
        segs.append(out)
    set_id = [0 if j in SET0_JS else 1 for j in range(NJ)]
    act_set = [act_tiles_of_set(s) for s in set_id]
    nseg = sum(len(s) for s in segs)
    return segs, act_set, set_id, nseg


def plan_slots(segs):
    slot = {}
    s = 0
    for j in range(NJ):
        for si in range(len(segs[j])):
            slot[(j, si)] = s
            s += 1
    return slot


def len_dve_table(segs, act_set):
    """len_dve[t, k]: total {0,1}-segment length of class k for tile t."""
    ld = np.zeros((NT, K), np.float64)
    for j in range(NJ):
        for (a, b, cls) in segs[j]:
            for t in range(NT):
                if t not in act_set[j]:
                    ld[t, cls] += b - a
    return ld


def build_nc(cuts):
    segs, act_set, set_id, nseg = make_plan(cuts)
    slot = plan_slots(segs)

    nc = bacc.Bacc()
    featT = nc.dram_tensor("featT", [C, N], FP16, kind="ExternalInput")
    projwT = nc.dram_tensor("projwT", [C, DLOC], FP16, kind="ExternalInput")
    # cwb = 2*cw - len_dve, laid out [PT, (t k)]
    cwb = nc.dram_tensor("cwb", [PT, NT * K], mybir.dt.float32, kind="ExternalInput")
    p_out = nc.dram_tensor("p_out", [4 * K, N], mybir.dt.float32, kind="ExternalOutput")
    # u2 = cwb + sA + 2*sB, fp32 -> host recovers S = u2 - 2cw
    u2_out = nc.dram_tensor("u2_out", [PT, NT * K], mybir.dt.float32, kind="ExternalOutput")

    with tile.TileContext(nc) as tc:
        with (
            tc.tile_pool(name="singles", bufs=1) as singles,
            tc.tile_pool(name="feat", bufs=3) as featp,
            tc.tile_pool(name="pstage", bufs=4) as pstp,
        ):
            # hv tiles first so their SBUF byte offsets stay 16B-aligned
            # (DoubleRow rhs requires 2B-aligned partition addresses)
            hv = [singles.tile([PT, 2, N], FP8, name=f"hv{p}") for p in range(NPAIR)]
            projw_sb = singles.tile([C, DLOC], FP16)
            # first projw tile + feat half first so mm1(0,0) starts early
            fj0 = featp.tile([C, SCH], FP16, tag="fj")
            nc.gpsimd.dma_start(out=projw_sb[:, :PT], in_=projwT[:, :PT])
            nc.sync.dma_start(out=fj0[:, :MCH], in_=featT[:, :MCH])
            nc.gpsimd.dma_start(out=projw_sb[:, PT:], in_=projwT[:, PT:])
            nc.sync.dma_start(out=fj0[:, MCH:], in_=featT[:, MCH:SCH])
            # 2D tiles for DMA; 3D/4D rearranged views for strided compute
            # (flatten()/multi-dim DMA APs fail NEFF load)
            cwb2 = singles.tile([PT, NT * K], mybir.dt.float32)
            u2d2 = singles.tile([PT, NT * K], mybir.dt.float32)
            u2f2 = singles.tile([PT, NT * K], mybir.dt.float32)
            cwb3 = cwb2.rearrange("p (t k) -> p t k", t=NT)
            u2d3 = u2d2.rearrange("p (t k) -> p t k", t=NT)
            u2f3 = u2f2.rearrange("p (t k) -> p t k", t=NT)
            spart = singles.tile([PT, nseg * NT], mybir.dt.float32)
            # stationaries: [PT, half, pair, 16] fp8 view; cols 0:3 A-hi,
            # 3:6 A-lo, 6:9 B-hi, 9:12 B-lo, 12:16 zero pad.  mm2 lhsT slice
            # stat4[:, :, p, 0:12] has outer free step 80B (16B-aligned).
            stat2 = [singles.tile([PT, 2 * NPAIR * 16], FP8, name=f"st{s}")
                     for s in range(2)]
            stat = [s2.rearrange("p (h q c) -> p h q c", h=2, q=NPAIR)
                    for s2 in stat2]
            dums = singles.tile([C, 256], mybir.dt.bfloat16)
            nc.gpsimd.dma_start(out=cwb2, in_=cwb[:, :])
            nc.vector.memset(u2d2, 0.0)
            for s in range(2):
                nc.vector.memset(stat2[s], 0.0)
            nc.vector.memset(dums, 0.0)

            def emit_groups(j):
                """Pool strided group-adds for superchunk j's accum slots
                (+-1 sums -> cwb3, {0,1} counts -> u2d3)."""
                for si in range(len(segs[j])):
                    s = slot[(j, si)]
                    cls = segs[j][si][2]
                    base = s * NT
                    if set_id[j] == 1:  # ACT = evens, DVE = odds
                        nc.gpsimd.tensor_tensor(
                            cwb3[:, 0:NT:2, cls], cwb3[:, 0:NT:2, cls],
                            spart[:, base: base + NT: 2], mybir.AluOpType.add)
                        nc.gpsimd.tensor_tensor(
                            u2d3[:, 1:NT:2, cls], u2d3[:, 1:NT:2, cls],
                            spart[:, base + 1: base + NT: 2], mybir.AluOpType.add)
                    else:               # ACT = {0,2,4,6}, DVE = odds + {8}
                        nc.gpsimd.tensor_tensor(
                            cwb3[:, 0:NT - 2:2, cls], cwb3[:, 0:NT - 2:2, cls],
                            spart[:, base: base + NT - 2: 2], mybir.AluOpType.add)
                        nc.gpsimd.tensor_tensor(
                            u2d3[:, 1:NT:2, cls], u2d3[:, 1:NT:2, cls],
                            spart[:, base + 1: base + NT: 2], mybir.AluOpType.add)
                        nc.gpsimd.tensor_tensor(
                            u2d3[:, NT - 2, cls: cls + 1],
                            u2d3[:, NT - 2, cls: cls + 1],
                            spart[:, base + NT - 2: base + NT - 1],
                            mybir.AluOpType.add)

            # ---- produce: z psum chunks -> fp8 hv tiles + segment sums ----
            with tc.tile_pool(name="mm1ps", bufs=4, space=MemorySpace.PSUM) as mm1ps:
                for j in range(NJ):
                    if j == 0:
                        fj = fj0
                    else:
                        fj = featp.tile([C, SCH], FP16, tag="fj")
                        dma_eng = nc.gpsimd if j % 2 == 0 else nc.sync
                        dma_eng.dma_start(
                            out=fj, in_=featT[:, j * SCH:(j + 1) * SCH])
                    if j >= 2:
                        emit_groups(j - 2)
                    for t in range(NT):
                        ps = mm1ps.tile([PT, SCH], mybir.dt.float32, tag="mm1")
                        for h in range(SCH // MCH):
                            nc.tensor.matmul(
                                ps[:, h * MCH:(h + 1) * MCH],
                                projw_sb[:, t * PT:(t + 1) * PT],
                                fj[:, h * MCH:(h + 1) * MCH],
                                start=True, stop=True,
                            )
                        on_act = t in act_set[j]
                        for si, (s0, s1, _cls) in enumerate(segs[j]):
                            hv_sl = hv[t // 2][:, t % 2, j * SCH + s0: j * SCH + s1]
                            col = slot[(j, si)] * NT + t
                            acc = spart[:, col: col + 1]
                            if on_act:
                                nc.scalar.activation(
                                    hv_sl, ps[:, s0:s1],
                                    mybir.ActivationFunctionType.Sign,
                                    accum_out=acc,
                                )
                            else:
                                # {0,1} in one op; accum = count of positives
                                # (op1 is the accum reduce op, not elementwise)
                                nc.vector.tensor_scalar(
                                    hv_sl, ps[:, s0:s1], 0.0, None,
                                    mybir.AluOpType.is_ge, mybir.AluOpType.add,
                                    accum_out=acc,
                                )

            # ---- finalize: u2 = cwb + sA + 2*sB; fp8 hi/lo into both sets --
            emit_groups(NJ - 2)
            emit_groups(NJ - 1)
            nc.gpsimd.tensor_tensor(u2f2, cwb2, u2d2, mybir.AluOpType.add)
            nc.gpsimd.tensor_tensor(u2f2, u2f2, u2d2, mybir.AluOpType.add)
            nc.gpsimd.dma_start(out=u2_out[:, :], in_=u2f2)

            # wide fp8 casts: hi = fp8(u2/2) on ACT, lo = fp8(u2/2 - hi) on DVE
            Copy = mybir.ActivationFunctionType.Copy

            def cast_hi_lo(st, h, psl, u2sl, in_a):
                b = 0 if in_a else 2 * K
                hi = st[:, h, psl, b: b + K]
                lo = st[:, h, psl, b + K: b + 2 * K]
                nc.scalar.activation(hi, u2sl, Copy, scale=0.5)
                nc.vector.scalar_tensor_tensor(
                    lo, u2sl, 0.5, hi,
                    mybir.AluOpType.mult, mybir.AluOpType.subtract,
                )

            # pattern 0 (S4): A = {0,2,4,6}; B = {1,3,5,7} + {8} + {9}
            cast_hi_lo(stat[0], 0, slice(0, 4), u2f3[:, 0:NT - 2:2, :], True)
            cast_hi_lo(stat[0], 1, slice(0, 4), u2f3[:, 1:NT - 1:2, :], False)
            cast_hi_lo(stat[0], 0, 4, u2f3[:, NT - 2, :], False)
            cast_hi_lo(stat[0], 1, 4, u2f3[:, NT - 1, :], False)
            # pattern 1 (S_EVEN): A = evens; B = odds
            cast_hi_lo(stat[1], 0, slice(0, NPAIR), u2f3[:, 0:NT:2, :], True)
            cast_hi_lo(stat[1], 1, slice(0, NPAIR), u2f3[:, 1:NT:2, :], False)

            with tc.tile_pool(name="pps", bufs=GRP2, space=MemorySpace.PSUM) as pps:
                # ---- PE keep-warm bridge over the drain/finalize tail ------
                for i in range(N_DUMMY):
                    dpp = pps.tile([128, MCH2], mybir.dt.float32, tag="pp")
                    nc.tensor.matmul(
                        dpp[:PT, :256], dums[:, 0:PT], dums,
                        start=True, stop=True,
                    )

                # ---- mm2: P2 partials via fp8 DoubleRow, pair-outer so the
                # stationary (and its LDWEIGHTS) is reused across the group's
                # 8 chunks; chunks accumulate in psum across the pair loop.
                for g in range(NC2 // GRP2):
                    pts = [pps.tile([128, MCH2], mybir.dt.float32, tag="pp",
                                    name=f"pp{g}_{ci}")
                           for ci in range(GRP2)]
                    for p in range(NPAIR):
                        for ci in range(GRP2):
                            c = g * GRP2 + ci
                            sid = set_id[(c * MCH2) // SCH]
                            nc.tensor.matmul(
                                pts[ci][:4 * K, :], stat[sid][:, :, p, 0:4 * K],
                                hv[p][:, :, c * MCH2:(c + 1) * MCH2],
                                start=(p == 0), stop=(p == NPAIR - 1),
                                perf_mode=mybir.MatmulPerfMode.DoubleRow,
                                skip_group_check=True,
                            )
                    pst = None
                    for ci in range(GRP2):
                        c = g * GRP2 + ci
                        if ci % 2 == 0:
                            pst = pstp.tile([4 * K, 2 * MCH2], mybir.dt.float32,
                                            tag="pst")
                        half = pst[:, (ci % 2) * MCH2:(ci % 2 + 1) * MCH2]
                        # only ACT/DVE can read PSUM
                        if ci % 2 == 0:
                            nc.vector.tensor_copy(half, pts[ci][:4 * K, :])
                        else:
                            nc.scalar.activation(half, pts[ci][:4 * K, :], Copy)
                        if ci % 2 == 1:
                            dma_eng = nc.gpsimd if (ci // 2) % 2 == 0 else nc.sync
                            dma_eng.dma_start(
                                out=p_out[:, (c - 1) * MCH2:(c + 1) * MCH2],
                                in_=pst,
                            )
    nc.compile()
    return nc


def _prep_inputs(feat_s, proj_w, classify_weights, cuts):
    segs, act_set, _set_id, _nseg = make_plan(cuts)
    ld = len_dve_table(segs, act_set)

    featT = np.ascontiguousarray(feat_s.T).astype(np.float16)  # [128, N]
    in_maps = []
    for core in range(NCORES):
        sl = slice(core * DLOC, (core + 1) * DLOC)
        projwT = np.ascontiguousarray(proj_w[sl].T).astype(np.float16)  # [128, DLOC]
        cw2 = classify_weights[:, sl].astype(np.float64).T              # [DLOC, 3]
        cw2 = 2.0 * cw2.reshape(NT, PT, K) - ld[:, None, :]
        cwbv = np.ascontiguousarray(
            cw2.transpose(1, 0, 2).reshape(PT, NT * K)
        ).astype(np.float32)
        in_maps.append({"featT": featT, "projwT": projwT, "cwb": cwbv})
    return in_maps


def _fp8_round(x):
    import ml_dtypes
    return x.astype(ml_dtypes.float8_e4m3fn).astype(np.float64)


def kernel(feat, proj_w, classify_weights, labels, _trace=False):
    global LAST_RESULTS
    feat = np.asarray(feat, dtype=np.float32)
    proj_w = np.asarray(proj_w, dtype=np.float32)
    classify_weights = np.asarray(classify_weights, dtype=np.float32)
    labels = np.asarray(labels).astype(np.int64)

    perm = np.argsort(labels, kind="stable")
    feat_s = feat[perm]
    counts = np.bincount(labels, minlength=K)
    cuts = [int(counts[0]), int(counts[0] + counts[1])]

    nc = build_nc(cuts)
    in_maps = _prep_inputs(feat_s, proj_w, classify_weights, cuts)
    res = run_bass_kernel_spmd(nc, in_maps, list(range(NCORES)), trace=_trace)
    LAST_RESULTS = res

    _segs, _act_set, set_id, _nseg = make_plan(cuts)

    # device u2 (fp32, exactly what the stationaries were quantized from)
    u2dev = np.zeros((K, D), np.float64)
    for core in range(NCORES):
        u2raw = np.asarray(res.results[core]["u2_out"]).astype(np.float64)
        u2raw = u2raw.reshape(PT, NT, K)
        for t in range(NT):
            dsl = slice(core * DLOC + t * PT, core * DLOC + (t + 1) * PT)
            u2dev[:, dsl] = u2raw[:, t, :].T

    S = u2dev - 2.0 * classify_weights.astype(np.float64)
    updated = classify_weights.astype(np.float64) + LAM * S
    norms = np.linalg.norm(updated, axis=1)

    # quantized u2/2 exactly as the device stationaries hold it
    hi = _fp8_round(u2dev / 2.0)
    q = hi + _fp8_round(u2dev / 2.0 - hi)

    # per-pattern {0,1}-block colsum corrections
    b_tiles = [tuple(t for t in range(NT) if t not in act_tiles_of_set(s))
               for s in range(2)]
    csB = np.zeros((2, K), np.float64)
    for s in range(2):
        for core in range(NCORES):
            for t in b_tiles[s]:
                dsl = slice(core * DLOC + t * PT, core * DLOC + (t + 1) * PT)
                csB[s] += 2.0 * q[:, dsl].sum(axis=1)

    hvu2 = np.zeros((K, N), np.float64)
    for core in range(NCORES):
        p = np.asarray(res.results[core]["p_out"]).astype(np.float64)   # [12, N]
        hvu2 += 2.0 * (p[0:3] + p[3:6]) + 4.0 * (p[6:9] + p[9:12])
    for c in range(NC2):
        sid = set_id[(c * MCH2) // SCH]
        hvu2[:, c * MCH2:(c + 1) * MCH2] -= csB[sid][:, None]

    logits_sorted = (hvu2 / (2.0 * np.maximum(norms, EPS))[:, None]).T.astype(np.float32)
    out = np.empty((N, K), np.float32)
    out[perm] = logits_sorted
    return out
